# revision 73
# baseline (speedup 1.0000x reference)
"""DynamicConv Trainium2 kernel.

Problem: x[32,256,64,64] f32. Attention branch (GAP -> FC(64) -> ReLU ->
FC(4) -> softmax) yields per-batch weights attn[b, k] over K=4 depthwise
3x3 kernels; output = sum_k attn[b,k] * depthwise_conv(x, kernel_k).

Algorithm used here (4x less conv work than the reference formulation):
the conv is linear in the kernel taps, so combine the K kernels first:
    w_eff[b,c,dy,dx] = sum_k attn[b,k] * conv_w[k,c,0,dy,dx]
then do ONE depthwise 3x3 conv per image with per-(b,c) taps.

Mapping (per NeuronCore, data-parallel over batch, 4 images/core):
  - channels on partitions (2 groups of 128), pixels on the free dim.
  - 7 of the 9 taps run on the TensorEngine as diagonal matmuls:
    lhsT = diag(w_eff[:,tap]) (float32r, 1 col/cycle), rhs = shifted view
    of the x tile; the 9-tap sum accumulates natively in PSUM per
    512-pixel bank. Diagonal matrices are built by bouncing w_eff through
    a pre-zeroed DRAM buffer (DRAM is linear => the diagonal is a single
    uniform-stride DMA).
  - center tap (0,0) runs on ScalarE (activation Copy, per-partition
    scale), tap (0,1) on VectorE (scalar_tensor_tensor fused MAC).
  - VectorE merges PSUM + the SBUF partial and writes the output tile
    (PSUM is not DMA-able).
  - GAP runs on ScalarE (Copy activation with accum_out), the attention
    MLP on PE (with fc2 bias folded into an augmented weight row), and
    the whole attention pipeline for batch b+1 is software-pipelined
    under batch b's conv.
"""

from contextlib import ExitStack

import numpy as np

B_FULL, C, H, W = 32, 256, 64, 64
K, KS, RED = 4, 3, 4
N_CORES = 8
B_LOC = B_FULL // N_CORES  # 4 images per core
NG = C // 128              # 2 channel groups of 128 partitions
HW = H * W                 # 4096 pixels
NBANKS = 8                 # 512-pixel PSUM banks per image
ROWS_PER_BANK = H // NBANKS  # 8 image rows per bank

TAPS = [(dy, dx) for dy in (-1, 0, 1) for dx in (-1, 0, 1)]
ACT_TAP = (0, 0)   # full coverage -> ScalarE write tap
DVE_TAP = (0, 1)   # VectorE fused-MAC tap
PE_TAPS = [t for t in TAPS if t not in (ACT_TAP, DVE_TAP)]


def tap_idx(dy, dx):
    return (dy + 1) * 3 + (dx + 1)


def build_bass():
    import concourse.bacc as bacc
    import concourse.bass as bass
    import concourse.tile as tile
    from concourse import mybir

    f32 = mybir.dt.float32
    f32r = mybir.dt.float32r

    nc = bacc.Bacc("TRN2", target_bir_lowering=False)

    x_d = nc.dram_tensor("x", [B_LOC, C, H, W], f32, kind="ExternalInput")
    convw_d = nc.dram_tensor("conv_w", [K, C, 1, KS, KS], f32, kind="ExternalInput")
    fc1w_d = nc.dram_tensor("fc1_w", [C // RED, C], f32, kind="ExternalInput")
    fc1b_d = nc.dram_tensor("fc1_b", [C // RED], f32, kind="ExternalInput")
    fc2w_d = nc.dram_tensor("fc2_w", [K, C // RED], f32, kind="ExternalInput")
    fc2b_d = nc.dram_tensor("fc2_b", [K], f32, kind="ExternalInput")
    out_d = nc.dram_tensor("out", [B_LOC, C, H, W], f32, kind="ExternalOutput")

    DH = C // RED  # 64 hidden units

    with tile.TileContext(nc) as tc, ExitStack() as ctx:
        singles = ctx.enter_context(tc.tile_pool(name="singles", bufs=1))
        xin = ctx.enter_context(tc.tile_pool(name="xin", bufs=5))
        partials = ctx.enter_context(tc.tile_pool(name="partials", bufs=2))
        outs = ctx.enter_context(tc.tile_pool(name="outs", bufs=2))
        diags = ctx.enter_context(tc.tile_pool(name="diags", bufs=4))
        smalls = ctx.enter_context(tc.tile_pool(name="smalls", bufs=3))
        dram = ctx.enter_context(tc.tile_pool(name="dram", bufs=1, space="DRAM"))
        cpsum = ctx.enter_context(tc.tile_pool(name="cpsum", bufs=7, space="PSUM"))
        mpsum = ctx.enter_context(tc.tile_pool(name="mpsum", bufs=1, space="PSUM"))

        bf16 = mybir.dt.bfloat16
        NPT = len(PE_TAPS)  # 7
        gapscr = singles.tile([128, HW], f32, tag="gapscr")

        diag_load_insts = {}
        x_load_insts = {}

        def emit_load(b, halves=False):
            x_t = []
            for g in range(NG):
                t = xin.tile([128, HW], f32r, tag="x", name=f"x_{b}_{g}")
                src_ap = x_d[b, g * 128:(g + 1) * 128, :, :].bitcast(f32r)
                src_fl = src_ap.rearrange("p h w -> p (h w)")
                if halves:
                    for q in range(2):
                        lo = q * (HW // 2)
                        li = nc.sync.dma_start(out=t[:, lo:lo + HW // 2],
                                               in_=src_fl[:, lo:lo + HW // 2])
                        x_load_insts.setdefault(b, []).append(li)
                else:
                    li = nc.sync.dma_start(out=t[:], in_=src_ap)
                    x_load_insts.setdefault(b, []).append(li)
                x_t.append(t)
            return x_t

        # ---- DRAM diag bounce buffer (pre-zeroed) ---------------------------
        # Only the 7 PE taps need diagonals. layout: [b%2][grp][slot<7]
        # [k=128][m=128] f32; diag elem (c,c) of slot s at element offset
        # (par*NG+grp)*7*16384 + s*16384 + c*129. Double-buffered on batch
        # parity so batch b+1's scatter never races batch b's diag load.
        # Parity 0 is zeroed FIRST (on the fast HWDGE path, ahead of the x
        # loads): batch 0's scatter gates on it, and it is the binding
        # constraint of the whole prologue if it queues behind x.
        diag_dram = dram.tile([2 * NG * NPT * 128 * 128], f32)
        zhalf = NG * NPT * 64  # half the per-parity region per partition
        zt = singles.tile([128, zhalf], f32, tag="zeros")
        nc.gpsimd.memset(zt[:], 0.0)

        def emit_zero(par):
            for h in range(2):
                nc.sync.dma_start(
                    out=bass.AP(tensor=diag_dram.tensor,
                                offset=(diag_dram.offset
                                        + par * NG * NPT * 16384
                                        + h * 128 * zhalf),
                                ap=[[zhalf, 128], [1, zhalf]]),
                    in_=zt[:],
                )

        emit_zero(0)

        # x(0) heads the rest of the DMA queue: everything else in the
        # prologue chain depends on it.
        x0 = emit_load(0, halves=True)

        # ---- static weights -------------------------------------------------
        # fc2_wT augmented with a bias row: [h (partitions) + 1, k]; row DH
        # holds fc2_b, and h_aug = [relu(h); 1] folds the bias into the MM.
        fc2wT = singles.tile([DH + 1, K], f32, tag="fc2wT")
        nc.sync.dma_start(
            out=fc2wT[:DH, :],
            in_=bass.AP(tensor=fc2w_d, offset=0, ap=[[1, DH], [DH, K]]),
        )
        nc.sync.dma_start(out=fc2wT[DH:DH + 1, :],
                          in_=bass.AP(tensor=fc2b_d, offset=0,
                                      ap=[[K, 1], [1, K]]))

        fc1b = singles.tile([DH, 1], f32, tag="fc1b")
        nc.sync.dma_start(out=fc1b[:], in_=fc1b_d[:].unsqueeze(1))

        # conv_w per (k, grp): [c (partitions), 9 taps]
        convw_sb = [[None] * K for _ in range(NG)]
        for g in range(NG):
            for k in range(K):
                t = singles.tile([128, KS * KS], f32, tag=f"cw{g}_{k}")
                src = bass.AP(tensor=convw_d,
                              offset=k * C * KS * KS + g * 128 * KS * KS,
                              ap=[[KS * KS, 128], [1, KS * KS]])
                nc.sync.dma_start(out=t[:], in_=src)
                convw_sb[g][k] = t

        # fc1_wT[grp]: [c within group (partitions), m] = fc1_w[m, c] / HW
        # (the 1/HW folds the GAP mean into fc1; fc1_b is added after, so
        #  this matches relu(mean(x) @ fc1_w.T + fc1_b)).
        # fc1_w is loaded CONTIGUOUSLY (a strided 4-byte transpose-gather
        # DMA costs ~3.6us and gates the whole prologue attention chain)
        # and transposed on-chip via PE with an affine_select identity.
        ones64 = singles.tile([DH, DH], f32, tag="ones64")
        nc.gpsimd.memset(ones64[:], 1.0)
        ident64 = singles.tile([DH, DH], f32, tag="ident64")
        nc.gpsimd.affine_select(
            out=ident64[:], in_=ones64[:], pattern=[[-1, DH]],
            compare_op=mybir.AluOpType.is_equal, fill=0.0,
            base=0, channel_multiplier=1)

        fc1w_sb = singles.tile([DH, C], f32, tag="fc1w_sb")
        nc.sync.dma_start(out=fc1w_sb[:], in_=fc1w_d[:])

        fc1wT = []
        for g in range(NG):
            tps = mpsum.tile([128, DH], f32, tag="mlp")
            nc.tensor.transpose(tps[:], fc1w_sb[:, g * 128:(g + 1) * 128],
                                ident64[:])
            t = singles.tile([128, DH], f32, tag=f"fc1wT{g}")
            nc.scalar.mul(t[:], tps[:], 1.0 / HW)
            fc1wT.append(t)

        def emit_gap(b, x_t):
            """Per-channel spatial sums for batch b.

            On ScalarE (the Copy activation's accum_out yields the sum) so
            VectorE stays clear for conv merges. The prologue batch splits
            the work across DVE and ACT per half-tile for latency.
            """
            gsum = smalls.tile([128, NG], f32, tag="gsum", name=f"gsum_{b}")
            if b == 0:
                # prologue: one reduce per group, DVE and ACT in parallel
                nc.vector.tensor_reduce(
                    out=gsum[:, 0:1], in_=x_t[0][:].bitcast(f32),
                    axis=mybir.AxisListType.X, op=mybir.AluOpType.add)
                nc.scalar.activation(gapscr[:], x_t[1][:].bitcast(f32),
                                     mybir.ActivationFunctionType.Copy,
                                     bias=0.0, scale=1.0,
                                     accum_out=gsum[:, 1:2])
            else:
                for g in range(NG):
                    nc.scalar.activation(gapscr[:], x_t[g][:].bitcast(f32),
                                         mybir.ActivationFunctionType.Copy,
                                         bias=0.0, scale=1.0,
                                         accum_out=gsum[:, g:g + 1])
            return gsum

        def emit_attention(b, gsum):
            """MLP -> softmax -> w_eff -> diag tiles for batch b.

            All small elementwise work runs on GpSimd (and one reciprocal on
            DVE) so neither the conv merges (DVE) nor the center taps (ACT)
            ever wait behind this chain.
            """
            h_ps = mpsum.tile([DH, 1], f32, tag="mlp")
            for g in range(NG):
                nc.tensor.matmul(h_ps[:], fc1wT[g][:], gsum[:, g:g + 1],
                                 start=(g == 0), stop=(g == NG - 1))
            # h_aug = [relu(h); 1] so the fc2 matmul folds in fc2_b.
            h_sb = smalls.tile([DH + 1, 1], f32, tag="h_sb")
            nc.scalar.activation(h_sb[:DH], h_ps[:],
                                 mybir.ActivationFunctionType.Relu,
                                 bias=fc1b[:], scale=1.0)
            nc.vector.memset(h_sb[DH:DH + 1, :], 1.0)

            a_ps = mpsum.tile([1, K], f32, tag="mlp")
            nc.tensor.matmul(a_ps[:], h_sb[:], fc2wT[:], start=True, stop=True)

            # softmax without the max-subtraction: the logits here are O(1)
            # (h and fc2_w are small), so exp cannot overflow and
            # softmax(x) == exp(x)/sum(exp(x)) exactly matches the reference.
            # The Exp's accum_out directly yields the softmax denominator.
            expv = smalls.tile([1, K], f32, tag="expv")
            ssum = smalls.tile([1, 1], f32, tag="ssum")
            nc.scalar.activation(expv[:], a_ps[:],
                                 mybir.ActivationFunctionType.Exp,
                                 bias=0.0, scale=1.0, accum_out=ssum[:])
            rsum = smalls.tile([1, 1], f32, tag="rsum")
            nc.vector.reciprocal(rsum[:], ssum[:])

            e_bc = smalls.tile([128, K], f32, tag="e_bc")
            nc.gpsimd.partition_broadcast(e_bc[:], expv[:])
            r_bc = smalls.tile([128, 1], f32, tag="r_bc")
            nc.gpsimd.partition_broadcast(r_bc[:], rsum[:])

            par = b % 2
            weff, negw, diag_sb = [], [], []
            for g in range(NG):
                wt = smalls.tile([128, KS * KS], f32, tag=f"weff{g}")
                nc.vector.tensor_scalar_mul(wt[:], convw_sb[g][0][:],
                                            e_bc[:, 0:1])
                for k in range(1, K):
                    nc.vector.scalar_tensor_tensor(
                        out=wt[:], in0=convw_sb[g][k][:],
                        scalar=e_bc[:, k:k + 1], in1=wt[:],
                        op0=mybir.AluOpType.mult, op1=mybir.AluOpType.add)
                nc.vector.tensor_scalar_mul(wt[:], wt[:], r_bc[:])
                weff.append(wt)
                nt = smalls.tile([128, KS * KS], f32, tag=f"negw{g}")
                nc.vector.tensor_scalar_mul(nt[:], wt[:], -1.0)
                negw.append(nt)
                # scatter PE taps onto the DRAM diagonal; slots 0-3
                # are w_eff cols 0-3, slots 4-6 are cols 6-8 (two runs).
                base = diag_dram.offset + (par * NG + g) * NPT * 16384
                nc.sync.dma_start(
                    out=bass.AP(tensor=diag_dram.tensor, offset=base,
                                ap=[[129, 128], [16384, 4]]),
                    in_=wt[:, 0:4],
                )
                nc.sync.dma_start(
                    out=bass.AP(tensor=diag_dram.tensor,
                                offset=base + 4 * 16384,
                                ap=[[129, 128], [16384, 3]]),
                    in_=wt[:, 6:9],
                )
                dt_ = diags.tile([128, NPT, 128], f32r, tag="diag",
                                 name=f"diag_{b}_{g}")
                di = nc.sync.dma_start(
                    out=dt_[:],
                    in_=bass.AP(tensor=diag_dram.tensor, offset=base,
                                ap=[[128, 128], [16384, NPT],
                                    [1, 128]]).bitcast(f32r),
                )
                diag_load_insts.setdefault(b, []).append(di)
                diag_sb.append(dt_)
            return weff, negw, diag_sb

        def emit_part_stage(b, x_t, weff, negw):
            """SBUF partial per group: ScalarE center tap + DVE edge fix-ups.

            The PE taps run on the FLAT image: out_flat[i] += w*x_flat[i+S],
            S = 64*dy + dx (fully contiguous => legal 2D matmul APs), with
            ranges rounded inward to even boundaries (f32r matmuls need even
            PSUM offset/size). The partial pre-subtracts the row-wrap terms
            the flat MMs wrongly add at one edge column per row, and adds
            back the 1-2 true edge contributions the even rounding dropped,
            so the PSUM+partial merge yields the exact zero-padded conv.
            """
            parts = []
            for g in range(NG):
                xt = x_t[g][:].bitcast(f32)
                x3 = xt.rearrange("p (h w) -> p h w", w=W)
                part = partials.tile([128, HW], f32, tag="part",
                                     name=f"part_{b}_{g}")
                p3 = part[:].rearrange("p (h w) -> p h w", w=W)
                # center tap on ScalarE: part = x * w[4]
                nc.scalar.activation(part[:], xt,
                                     mybir.ActivationFunctionType.Copy,
                                     bias=0.0, scale=weff[g][:, 4:5])
                for (dy, dx) in PE_TAPS:
                    if dx == 0:
                        continue
                    ti = tap_idx(dy, dx)
                    S = W * dy + dx
                    i0 = max(0, -S)
                    i1 = HW - max(0, S)
                    i0e = (i0 + 1) // 2 * 2
                    i1e = i1 // 2 * 2
                    # subtract row-wrap terms: p = 64h + e in [i0e, i1e)
                    e = 0 if dx < 0 else W - 1
                    s = W - 1 - e
                    d = dy + dx
                    h0 = -(-(i0e - e) // W)       # ceil div
                    h1 = (i1e - 1 - e) // W + 1
                    nc.vector.scalar_tensor_tensor(
                        out=p3[:, h0:h1, e:e + 1],
                        in0=x3[:, h0 + d:h1 + d, s:s + 1],
                        scalar=negw[g][:, ti:ti + 1],
                        in1=p3[:, h0:h1, e:e + 1],
                        op0=mybir.AluOpType.mult, op1=mybir.AluOpType.add)
                    # add back dropped true contributions
                    for p in ([i0] if i0e > i0 else []) + \
                             ([i1e] if i1 > i1e else []):
                        h, w_ = divmod(p, W)
                        if 0 <= h + dy < H and 0 <= w_ + dx < W:
                            nc.vector.scalar_tensor_tensor(
                                out=p3[:, h:h + 1, w_:w_ + 1],
                                in0=x3[:, h + dy:h + dy + 1,
                                       w_ + dx:w_ + dx + 1],
                                scalar=weff[g][:, ti:ti + 1],
                                in1=p3[:, h:h + 1, w_:w_ + 1],
                                op0=mybir.AluOpType.mult,
                                op1=mybir.AluOpType.add)
                parts.append(part)
            return parts

        QUAD = 4  # banks whose matmuls share one LDWEIGHTS per tap

        def emit_conv_stage(b, x_t, weff, diag_sb, parts, groups):
            for g in groups:
                xr = x_t[g][:]                      # [128, HW] float32r
                x3 = xr.bitcast(f32).rearrange("p (h w) -> p h w", w=W)
                part = parts[g]
                p3 = part[:].rearrange("p (h w) -> p h w", w=W)

                out_t = outs.tile([128, HW], f32, tag="out",
                                  name=f"out_{b}_{g}")
                for q0 in range(0, NBANKS, QUAD):
                    banks = range(q0, min(q0 + QUAD, NBANKS))
                    # (0,1) tap on VectorE per bank (the merges need it)
                    for j in banks:
                        r_base = j * ROWS_PER_BANK
                        nc.vector.scalar_tensor_tensor(
                            out=p3[:, r_base:r_base + ROWS_PER_BANK, 0:W - 1],
                            in0=x3[:, r_base:r_base + ROWS_PER_BANK, 1:W],
                            scalar=weff[g][:, 5:6],
                            in1=p3[:, r_base:r_base + ROWS_PER_BANK,
                                   0:W - 1],
                            op0=mybir.AluOpType.mult,
                            op1=mybir.AluOpType.add)

                    ps = {j: cpsum.tile([128, 512], f32, tag="cps",
                                        name=f"cps_{b}_{g}_{j}")
                          for j in banks}
                    # tap-major within the quad: the same diag slot feeds
                    # QUAD consecutive matmuls, so walrus emits one
                    # LDWEIGHTS per tap instead of one per matmul (the
                    # compiler runs with ldw-opt off). The first tap is
                    # full-coverage for every bank in the quad so it sets
                    # has_written across each bank.
                    first = (1, 0) if q0 == 0 else (-1, 0)
                    taps = [first] + [t for t in PE_TAPS if t != first]
                    for i, (dy, dx) in enumerate(taps):
                        S = W * dy + dx
                        t0 = max(0, -S)
                        t1 = HW - max(0, S)
                        for j in banks:
                            b0 = j * 512
                            i0 = max(b0, (t0 + 1) // 2 * 2)
                            i1 = min(b0 + 512, t1 // 2 * 2)
                            nc.tensor.matmul(
                                ps[j][:, i0 - b0:i1 - b0],
                                diag_sb[g][:, PE_TAPS.index((dy, dx)), :],
                                xr[:, i0 + S:i1 + S],
                                start=(i == 0), stop=(i == len(taps) - 1),
                                skip_group_check=True)
                    for j in banks:
                        b0 = j * 512
                        # merge PSUM + SBUF partial -> out (drains PSUM)
                        nc.vector.tensor_add(out_t[:, b0:b0 + 512],
                                             ps[j][:], part[:, b0:b0 + 512])
                    # drain the finished quad to DRAM; the very last quad
                    # of the kernel drains in small pieces so the final DMA
                    # (which nothing overlaps) is short.
                    lo = q0 * 512
                    span = QUAD * 512
                    last = (b == B_LOC - 1 and g == NG - 1
                            and q0 + QUAD >= NBANKS)
                    pieces = 4 if last else 1
                    for p_ in range(pieces):
                        plo = lo + p_ * (span // pieces)
                        nc.sync.dma_start(
                            out=out_d[b, g * 128:(g + 1) * 128, :, :]
                                .rearrange("p h w -> p (h w)")
                                [:, plo:plo + span // pieces],
                            in_=out_t[:, plo:plo + span // pieces])

        # Software pipeline, one batch ahead. Per-engine program order is
        # chosen so ScalarE runs batch b's center taps BEFORE batch b+1's
        # GAP copies (the merges depend on the center taps), and the
        # attention chain for b+1 completes while the PE is busy with b.
        x_tiles = {0: x0}
        gsums = {0: emit_gap(0, x_tiles[0])}
        stages = {0: emit_attention(0, gsums.pop(0))}
        if B_LOC > 1:
            x_tiles[1] = emit_load(1)
        # parity-1 zeroing is only needed by batch 1's scatter; emitting it
        # after x(1) keeps it off batch 0's diag-load critical path.
        emit_zero(1)

        for b in range(B_LOC):
            x_t = x_tiles.pop(b)
            weff, negw, diag_sb = stages.pop(b)
            parts = emit_part_stage(b, x_t, weff, negw)
            if b + 1 < B_LOC:
                gsums[b + 1] = emit_gap(b + 1, x_tiles[b + 1])
                stages[b + 1] = emit_attention(b + 1, gsums.pop(b + 1))
            if b + 2 < B_LOC:
                # x(b+2) is enqueued AFTER attention(b+1)'s diag loads so the
                # small diag DMAs never queue behind bulk x transfers.
                x_tiles[b + 2] = emit_load(b + 2)
            emit_conv_stage(b, x_t, weff, diag_sb, parts, groups=[0, 1])

    nc.compile()
    return nc


_COMPILED = None
LAST_RESULTS = None


def kernel(**inputs):
    global _COMPILED
    import concourse.mybir as mybir  # noqa: F401  (import side effects)
    from concourse.bass_utils import run_bass_kernel_spmd

    if _COMPILED is None:
        _COMPILED = build_bass()
    nc = _COMPILED

    x = np.ascontiguousarray(inputs["x"], dtype=np.float32)
    rep = {k: np.ascontiguousarray(v, dtype=np.float32)
           for k, v in inputs.items() if k != "x"}

    in_maps = []
    for i in range(N_CORES):
        m = {"x": np.ascontiguousarray(x[i * B_LOC:(i + 1) * B_LOC])}
        m.update(rep)
        in_maps.append(m)

    # the axon-tunneled PJRT execute can fail transiently; retry a couple
    # of times before giving up.
    last_exc = None
    for attempt in range(3):
        try:
            res = run_bass_kernel_spmd(nc, in_maps,
                                       core_ids=list(range(N_CORES)))
            break
        except Exception as e:  # noqa: BLE001
            last_exc = e
            import time
            time.sleep(2.0 * (attempt + 1))
    else:
        raise last_exc
    global LAST_RESULTS
    LAST_RESULTS = res
    return np.concatenate([r["out"] for r in res.results], axis=0)
